# revision 23
# baseline (speedup 1.0000x reference)
"""Trainium2 Bass kernel for the DUAN conditioned-normalization problem.

Contract: kernel(**inputs) takes FULL inputs (B=8 samples), shards one sample
per NeuronCore (8 cores), runs a single Bass/Tile kernel SPMD, and gathers the
full [8, 512, 8192] output.

Per-sample math (matches the jax reference):
  mu_c/var_c over L per channel; mu_l/var_l over (C,L);
  g = sigmoid(gw2 @ relu(gw1 @ c + gb1) + gb2); g_mix = mean_L(g)
  mu = g_mix*mu_c + (1-g_mix)*mu_l ; sigma likewise from sqrt(var+eps)
  gamma,beta = mw2 @ relu(mw1 @ mean_L(c) + mb1) + mb2
  y = (1+gamma)*(x-mu)/sigma + beta
  keep top-k channels by mean_L |y| (k=358), zero the rest.

V2 layout: x, c and the output travel as bf16 (24 MiB HBM traffic/core vs 40
in V1); x stays resident in SBUF.  Channel stats come from bn_stats split
across DVE and GPSIMD; the gate network runs on PE (bf16) + ACT sigmoid with
accumulator means; cond-pool is a DVE accumulating tensor_scalar over c; the
|y| importance pass is a DVE two-op chain (mult-add, abs_max+accum) with some
chunks on ACT; the top-k mask is computed via an exact PE fp32
transpose+broadcast and rank-by-count compare.  Output is (A*mask)*x+(B*mask)
streamed as bf16.
"""

import sys

sys.path.insert(0, "/opt/trn_rl_repo")

import numpy as np

B = 8
C = 512
L = 8192
H = 128
CG = 4           # channel groups of 128 partitions
SL = 1024        # phase-1 supertile width along L
NST = L // SL    # 8
LB = 512         # bn_stats chunk
NLB = L // LB    # 16
XC = 2048        # phase-2 chunk along L
NXC = L // XC    # 4
KEEP = max(1, int(C * 0.7))  # 358
EPS = 1e-5

# phase-2a split: ACT runs 4096-wide (g, jpair) ops; DVE runs 2048 chunks
ACT_2A_PAIRS = ((0, 0), (0, 2), (1, 0), (1, 2))
DVE_2A = ((2, 0), (2, 1), (2, 2), (2, 3), (3, 0), (3, 1), (3, 2), (3, 3))

_CACHE = {}


def _build_nc():
    import concourse.bacc as bacc
    import concourse.bass as bass
    import concourse.tile as tile
    from concourse import mybir

    f32 = mybir.dt.float32
    bf16 = mybir.dt.bfloat16
    AF = mybir.ActivationFunctionType
    OP = mybir.AluOpType
    AX = mybir.AxisListType

    nc = bacc.Bacc("TRN2", target_bir_lowering=False, debug=False, num_devices=8)

    x_d = nc.declare_dram_parameter("x", [C, L], bf16, isOutput=False)
    c_d = nc.declare_dram_parameter("c", [C, L], bf16, isOutput=False)
    gw1t_d = nc.declare_dram_parameter("gw1t", [128, CG, H], bf16, isOutput=False)
    gb1_d = nc.declare_dram_parameter("gb1c", [H, 1], f32, isOutput=False)
    gw2t_d = nc.declare_dram_parameter("gw2t", [H, C], bf16, isOutput=False)
    gb2_d = nc.declare_dram_parameter("gb2c", [H, CG], f32, isOutput=False)
    mw1t_d = nc.declare_dram_parameter("mw1t", [128, CG, H], bf16, isOutput=False)
    mb1_d = nc.declare_dram_parameter("mb1c", [H, 1], f32, isOutput=False)
    mw2t_d = nc.declare_dram_parameter("mw2t", [H, 2 * C], f32, isOutput=False)
    mb2_d = nc.declare_dram_parameter("mb2c", [H, 2 * CG], f32, isOutput=False)
    ident_d = nc.declare_dram_parameter("ident", [128, 128], f32, isOutput=False)
    out_d = nc.declare_dram_parameter("out", [C, L], bf16, isOutput=True)

    with tile.TileContext(nc) as tc:
        _emit(tc, bass, mybir, f32, bf16, AF, OP, AX,
              x_d, c_d, gw1t_d, gb1_d, gw2t_d, gb2_d,
              mw1t_d, mb1_d, mw2t_d, mb2_d, ident_d, out_d)

    nc.compile()
    return nc


def _emit(tc, bass, mybir, f32, bf16, AF, OP, AX,
          x_d, c_d, gw1t_d, gb1_d, gw2t_d, gb2_d,
          mw1t_d, mb1_d, mw2t_d, mb2_d, ident_d, out_d):
    from contextlib import ExitStack

    nc = tc.nc

    with ExitStack() as top:
        xpool = top.enter_context(tc.tile_pool(name="xbuf", bufs=1))
        wpool = top.enter_context(tc.tile_pool(name="wts", bufs=1))
        spool = top.enter_context(tc.tile_pool(name="stats", bufs=1))

        # ---- weights / biases into SBUF ----
        w1_sb = wpool.tile([128, CG, H], bf16, tag="w1", name="w1")
        nc.scalar.dma_start(out=w1_sb[:], in_=gw1t_d[:])
        m1_sb = wpool.tile([128, CG, H], bf16, tag="m1w", name="m1w")
        nc.scalar.dma_start(out=m1_sb[:], in_=mw1t_d[:])
        w2_sb = wpool.tile([128, C], bf16, tag="w2", name="w2")
        nc.scalar.dma_start(out=w2_sb[:], in_=gw2t_d[:])
        gb1_sb = wpool.tile([128, 1], f32, tag="gb1", name="gb1")
        nc.scalar.dma_start(out=gb1_sb[:], in_=gb1_d[:])
        gb2_sb = wpool.tile([128, CG], f32, tag="gb2", name="gb2")
        nc.scalar.dma_start(out=gb2_sb[:], in_=gb2_d[:])
        mb1_sb = wpool.tile([128, 1], f32, tag="mb1", name="mb1")
        nc.scalar.dma_start(out=mb1_sb[:], in_=mb1_d[:])
        mb2_sb = wpool.tile([128, 2 * CG], f32, tag="mb2", name="mb2")
        nc.scalar.dma_start(out=mb2_sb[:], in_=mb2_d[:])
        m2_sb = wpool.tile([128, 2 * C], f32, tag="m2w", name="m2w")
        nc.scalar.dma_start(out=m2_sb[:], in_=mw2t_d[:])
        ident_sb = wpool.tile([128, 128], f32, tag="ident", name="ident")
        nc.scalar.dma_start(out=ident_sb[:], in_=ident_d[:])

        ps_m = top.enter_context(tc.tile_pool(name="psm", bufs=1, space="PSUM"))
        m1_ps = ps_m.tile([128, LB], f32, tag="m1ps", name="m1ps")

        ones_sb = spool.tile([128, 128], f32, tag="ones", name="ones")
        nc.vector.memset(ones_sb[:], 1.0)

        # ---- persistent accumulators ----
        X_sb = [xpool.tile([128, L], bf16, tag=f"X{g}", name=f"X{g}") for g in range(CG)]
        stats = [spool.tile([128, NLB, 6], f32, tag=f"bnst{g}", name=f"bnst{g}")
                 for g in range(CG)]
        gacc = spool.tile([128, CG, NST], f32, tag="gacc", name="gacc")
        impacc = spool.tile([128, CG, NXC], f32, tag="impacc", name="impacc")
        nc.vector.memset(impacc[:], 0.0)
        muvar = spool.tile([128, CG, 2], f32, tag="muvar", name="muvar")
        work = spool.tile([128, 16], f32, tag="work", name="work")
        scal = spool.tile([128, 8], f32, tag="scal", name="scal")
        bl_sb = spool.tile([128, 2], f32, tag="blb", name="blb")
        gm4 = spool.tile([128, CG], f32, tag="gm4", name="gm4")
        mu4t = spool.tile([128, CG], f32, tag="mu4t", name="mu4t")
        sg4t = spool.tile([128, CG], f32, tag="sg4t", name="sg4t")
        imp4 = spool.tile([128, CG], f32, tag="imp4", name="imp4")
        A4 = spool.tile([128, CG], f32, tag="A4", name="A4")
        B4 = spool.tile([128, CG], f32, tag="B4", name="B4")
        A4m = spool.tile([128, CG], f32, tag="A4m", name="A4m")
        B4m = spool.tile([128, CG], f32, tag="B4m", name="B4m")
        rank4 = spool.tile([128, CG], f32, tag="rank4", name="rank4")
        mask4 = spool.tile([128, CG], f32, tag="mask4", name="mask4")
        hm_sb = spool.tile([128, 1], f32, tag="hm", name="hm")
        tr_sb = spool.tile([1, CG, 128], f32, tag="tr4", name="tr4")
        T_sb = spool.tile([128, C], f32, tag="Tsb", name="Tsb")
        G_sb = spool.tile([128, C], f32, tag="Gsb", name="Gsb")

        # =========================== phase 1 ===========================
        # 4 slots; each slot = one 2 MiB c DMA ([128,CG,XC], 3D AP) + four
        # 512 KiB x chunk DMAs, then gate compute for two SL supertiles and
        # raw x moments for the slot's chunk.
        with ExitStack() as ph1:
            cpool = ph1.enter_context(tc.tile_pool(name="cbuf", bufs=3))
            hpool = ph1.enter_context(tc.tile_pool(name="hbuf", bufs=2))
            gspool = ph1.enter_context(tc.tile_pool(name="gscr", bufs=4))
            ps_h = ph1.enter_context(tc.tile_pool(name="psh", bufs=1, space="PSUM"))
            ps_g = ph1.enter_context(tc.tile_pool(name="psg", bufs=2, space="PSUM"))

            for k in range(NXC):
                l0 = k * XC
                c_t = cpool.tile([128, CG, XC], bf16, tag="ct", name="ct")
                cap = c_d[:]
                c_src = bass.AP(tensor=cap.tensor, offset=l0,
                                ap=[[L, 128], [128 * L, CG], [1, XC]])
                nc.sync.dma_start(out=c_t[:], in_=c_src)
                for g in range(CG):
                    nc.gpsimd.dma_start(
                        out=X_sb[g][:, l0:l0 + XC],
                        in_=x_d[g * 128:(g + 1) * 128, l0:l0 + XC])

                for half in range(2):
                    st = k * 2 + half
                    # gate layer 1: h = relu(gw1 @ c + gb1)
                    h_ps = ps_h.tile([128, SL], f32, tag="hps", name="hps")
                    for g in range(CG):
                        for hh in range(2):
                            hs = slice(hh * LB, (hh + 1) * LB)
                            ds = slice(half * SL + hh * LB,
                                       half * SL + (hh + 1) * LB)
                            nc.tensor.matmul(h_ps[:, hs], w1_sb[:, g, :],
                                             c_t[:, g, ds],
                                             start=(g == 0), stop=(g == CG - 1))
                    h_sb = hpool.tile([128, SL], bf16, tag="hsb", name="hsb")
                    if st >= 4:
                        nc.scalar.activation(out=h_sb[:], in_=h_ps[:],
                                             func=AF.Relu,
                                             bias=gb1_sb[:, 0:1], scale=1.0)
                    else:
                        nc.vector.tensor_scalar(out=h_sb[:], in0=h_ps[:],
                                                scalar1=gb1_sb[:, 0:1],
                                                scalar2=0.0,
                                                op0=OP.add, op1=OP.max)

                    # cond-MLP layer 1: accumulate mw1 @ c into one PSUM bank
                    # (column j accumulates over all 512-blocks; reduced later)
                    for g in range(CG):
                        for hh in range(2):
                            ds = slice(half * SL + hh * LB,
                                       half * SL + (hh + 1) * LB)
                            first = (k == 0 and half == 0 and g == 0 and hh == 0)
                            last = (k == NXC - 1 and half == 1
                                    and g == CG - 1 and hh == 1)
                            nc.tensor.matmul(m1_ps[:], m1_sb[:, g, :],
                                             c_t[:, g, ds],
                                             start=first, stop=last)

                    # gate layer 2 + sigmoid (+ g_mix accumulator on ACT)
                    for g in range(CG):
                        g_ps = ps_g.tile([128, SL], f32, tag="gps", name="g_ps")
                        for hh in range(2):
                            hs = slice(hh * LB, (hh + 1) * LB)
                            nc.tensor.matmul(g_ps[:, hs],
                                             w2_sb[:, g * 128:(g + 1) * 128],
                                             h_sb[:, hs], start=True, stop=True)
                        g_scr = gspool.tile([128, SL], bf16, tag="gscr", name="gscr")
                        nc.scalar.activation(out=g_scr[:], in_=g_ps[:],
                                             func=AF.Sigmoid,
                                             bias=gb2_sb[:, g:g + 1], scale=1.0,
                                             accum_out=gacc[:, g, st:st + 1])

                # x channel stats: bn_stats per 512 chunk (DVE)
                for g in range(CG):
                    for hh in range(NLB // NXC):
                        j = k * (NLB // NXC) + hh
                        nc.vector.bn_stats(out=stats[g][:, j, :],
                                           in_=X_sb[g][:, j * LB:(j + 1) * LB])

        # =========================== finalize ===========================
        with ExitStack() as fin:
            ps_f = fin.enter_context(tc.tile_pool(name="psf", bufs=1, space="PSUM"))

            for g in range(CG):
                nc.vector.bn_aggr(out=muvar[:, g, :], in_=stats[g][:])
            mu_c = work[:, 0:4]
            var_c = muvar[:, :, 1]
            nc.vector.tensor_copy(out=mu_c, in_=muvar[:, :, 0])
            # work 4:8 = E[x^2] = var_c + mu_c^2
            nc.vector.tensor_tensor(out=work[:, 4:8], in0=mu_c, in1=mu_c,
                                    op=OP.mult)
            nc.vector.tensor_add(out=work[:, 4:8], in0=work[:, 4:8], in1=var_c)

            # cross-partition sums via ones-matmul -> [1, 8]
            colsum = ps_f.tile([128, 8], f32, tag="colsum", name="colsum")
            nc.tensor.matmul(colsum[0:1, :], ones_sb[:, 0:1], work[:, 0:8],
                             start=True, stop=True)

            # partition-0 scalars: mu_l, sigma_l
            nc.vector.reduce_sum(out=scal[0:1, 0:1], in_=colsum[0:1, 0:4], axis=AX.X)
            nc.vector.tensor_scalar(out=scal[0:1, 0:1], in0=scal[0:1, 0:1],
                                    scalar1=1.0 / C, scalar2=None, op0=OP.mult)
            nc.vector.reduce_sum(out=scal[0:1, 2:3], in_=colsum[0:1, 4:8], axis=AX.X)
            nc.vector.tensor_scalar(out=scal[0:1, 2:3], in0=scal[0:1, 2:3],
                                    scalar1=1.0 / C, scalar2=None, op0=OP.mult)
            nc.vector.tensor_tensor(out=scal[0:1, 3:4], in0=scal[0:1, 0:1],
                                    in1=scal[0:1, 0:1], op=OP.mult)
            nc.vector.tensor_tensor(out=scal[0:1, 1:2], in0=scal[0:1, 2:3],
                                    in1=scal[0:1, 3:4], op=OP.subtract)
            nc.vector.tensor_scalar(out=scal[0:1, 1:2], in0=scal[0:1, 1:2],
                                    scalar1=EPS, scalar2=None, op0=OP.add)
            nc.scalar.activation(out=scal[0:1, 1:2], in_=scal[0:1, 1:2],
                                 func=AF.Sqrt, bias=0.0, scale=1.0)

            # broadcast (mu_l, sigma_l) to all partitions
            bl_ps = ps_f.tile([128, 2], f32, tag="blps", name="blps")
            nc.tensor.matmul(bl_ps[:], ones_sb[0:1, :], scal[0:1, 0:2],
                             start=True, stop=True)
            nc.vector.tensor_copy(out=bl_sb[:], in_=bl_ps[:])
            mu_l = bl_sb[:, 0:1]
            sig_l = bl_sb[:, 1:2]

            # sigma_c = sqrt(var_c + eps)
            vpe4 = work[:, 8:12]
            sig4 = work[:, 12:16]
            nc.vector.tensor_scalar(out=vpe4, in0=var_c, scalar1=EPS,
                                    scalar2=None, op0=OP.add)
            nc.scalar.activation(out=sig4, in_=vpe4, func=AF.Sqrt,
                                 bias=0.0, scale=1.0)

            # g_mix
            nc.vector.tensor_reduce(out=gm4[:], in_=gacc[:], axis=AX.X, op=OP.add)
            nc.vector.tensor_scalar(out=gm4[:], in0=gm4[:], scalar1=1.0 / L,
                                    scalar2=None, op0=OP.mult)

            # mu = mu_l + g_mix*(mu_c - mu_l); sigma likewise
            nc.vector.tensor_scalar(out=mu4t[:], in0=mu_c, scalar1=mu_l,
                                    scalar2=None, op0=OP.subtract)
            nc.vector.tensor_tensor(out=mu4t[:], in0=mu4t[:], in1=gm4[:], op=OP.mult)
            nc.vector.tensor_scalar(out=mu4t[:], in0=mu4t[:], scalar1=mu_l,
                                    scalar2=None, op0=OP.add)
            nc.vector.tensor_scalar(out=sg4t[:], in0=sig4, scalar1=sig_l,
                                    scalar2=None, op0=OP.subtract)
            nc.vector.tensor_tensor(out=sg4t[:], in0=sg4t[:], in1=gm4[:], op=OP.mult)
            nc.vector.tensor_scalar(out=sg4t[:], in0=sg4t[:], scalar1=sig_l,
                                    scalar2=None, op0=OP.add)

            # cond MLP: hm = relu(mean_L(mw1 @ c) + mb1); m1_ps holds the
            # PSUM-accumulated mw1 @ c partial sums (columns sum to sum_L)
            nc.vector.reduce_sum(out=hm_sb[:], in_=m1_ps[:], axis=AX.X)
            nc.scalar.activation(out=hm_sb[:], in_=hm_sb[:], func=AF.Relu,
                                 bias=mb1_sb[:, 0:1], scale=1.0 / L)
            gb_ps = ps_f.tile([128, 2 * CG], f32, tag="gbps", name="gbps")
            for o in range(2 * CG):
                nc.tensor.matmul(gb_ps[:, o:o + 1],
                                 m2_sb[:, o * 128:(o + 1) * 128], hm_sb[:],
                                 start=True, stop=True)

            # A = (1+gamma)/sigma ; B = beta - mu*A
            inv4 = work[:, 8:12]
            nc.vector.reciprocal(out=inv4, in_=sg4t[:])
            gam4 = work[:, 12:16]
            nc.vector.tensor_add(out=gam4, in0=gb_ps[:, 0:CG], in1=mb2_sb[:, 0:CG])
            nc.vector.tensor_scalar(out=gam4, in0=gam4, scalar1=1.0,
                                    scalar2=None, op0=OP.add)
            bet4 = work[:, 4:8]
            nc.vector.tensor_add(out=bet4, in0=gb_ps[:, CG:2 * CG],
                                 in1=mb2_sb[:, CG:2 * CG])
            muA = work[:, 0:4]
            nc.vector.tensor_tensor(out=A4[:], in0=gam4, in1=inv4, op=OP.mult)
            nc.vector.tensor_tensor(out=muA, in0=mu4t[:], in1=A4[:], op=OP.mult)
            nc.vector.tensor_tensor(out=B4[:], in0=bet4, in1=muA, op=OP.subtract)

        # =========================== phase 2 ===========================
        with ExitStack() as ph2:
            ypool = ph2.enter_context(tc.tile_pool(name="ybuf", bufs=4))
            apool = ph2.enter_context(tc.tile_pool(name="abuf", bufs=2))
            ps_t = ph2.enter_context(tc.tile_pool(name="pst", bufs=1, space="PSUM"))

            # 2a: imp = sum_L |A*x + B| ; interleave ACT / DVE work
            alt = []
            for i in range(max(len(ACT_2A_PAIRS), len(DVE_2A))):
                if i < len(ACT_2A_PAIRS):
                    alt.append(("act",) + ACT_2A_PAIRS[i])
                if i < len(DVE_2A):
                    alt.append(("dve",) + DVE_2A[i])
            for eng, g, j in alt:
                if eng == "act":
                    xa = X_sb[g][:, j * XC:(j + 2) * XC]
                    scr = apool.tile([128, 2 * XC], bf16, tag="ascr", name="ascr")
                    nc.scalar.activation(out=scr[:], in_=xa, func=AF.Abs,
                                         bias=B4[:, g:g + 1],
                                         scale=A4[:, g:g + 1],
                                         accum_out=impacc[:, g, j:j + 1])
                else:
                    xa = X_sb[g][:, j * XC:(j + 1) * XC]
                    y_scr = apool.tile([128, XC], bf16, tag="yscr", name="y_scr")
                    nc.vector.tensor_scalar(out=y_scr[:], in0=xa,
                                            scalar1=A4[:, g:g + 1],
                                            scalar2=B4[:, g:g + 1],
                                            op0=OP.mult, op1=OP.add)
                    nc.vector.tensor_reduce(out=impacc[:, g, j:j + 1],
                                            in_=y_scr[:], axis=AX.X,
                                            op=OP.add, apply_absolute_value=True)
            nc.vector.tensor_reduce(out=imp4[:], in_=impacc[:], axis=AX.X, op=OP.add)

            # 2b: exact fp32 broadcast of imp via PE transpose + ones outer
            tr_ps = ps_t.tile([1, CG, 128], f32, tag="trps", name="trps")
            for g in range(CG):
                nc.tensor.matmul(tr_ps[0:1, g, :], imp4[:, g:g + 1],
                                 ident_sb[:], is_transpose=True,
                                 start=True, stop=True)
            nc.vector.tensor_copy(out=tr_sb[:], in_=tr_ps[:])
            T_ps = ps_t.tile([128, C], f32, tag="Tps", name="Tps")
            for g in range(CG):
                nc.tensor.matmul(T_ps[:, g * 128:(g + 1) * 128],
                                 ones_sb[0:1, 0:128], tr_sb[0:1, g, :],
                                 start=True, stop=True)
            nc.vector.tensor_copy(out=T_sb[:], in_=T_ps[:])
            for g in range(CG):
                nc.vector.tensor_scalar(out=G_sb[:], in0=T_sb[:],
                                        scalar1=imp4[:, g:g + 1], scalar2=0.0,
                                        op0=OP.is_gt, op1=OP.add,
                                        accum_out=rank4[:, g:g + 1])
            nc.vector.tensor_scalar(out=mask4[:], in0=rank4[:], scalar1=float(KEEP),
                                    scalar2=None, op0=OP.is_lt)
            nc.vector.tensor_tensor(out=A4m[:], in0=A4[:], in1=mask4[:], op=OP.mult)
            nc.vector.tensor_tensor(out=B4m[:], in0=B4[:], in1=mask4[:], op=OP.mult)

            # 2c: out = (A*mask)*x + (B*mask) -> bf16 -> HBM
            for idx in range(CG * NXC):
                g, j = divmod(idx, NXC)
                y_t = ypool.tile([128, XC], bf16, tag="yt", name="yt")
                xa = X_sb[g][:, j * XC:(j + 1) * XC]
                nc.vector.tensor_scalar(out=y_t[:], in0=xa,
                                        scalar1=A4m[:, g:g + 1],
                                        scalar2=B4m[:, g:g + 1],
                                        op0=OP.mult, op1=OP.add)
                nc.sync.dma_start(
                    out=out_d[g * 128:(g + 1) * 128, j * XC:(j + 1) * XC],
                    in_=y_t[:])


def _get_nc():
    if "nc" not in _CACHE:
        _CACHE["nc"] = _build_nc()
    return _CACHE["nc"]


def _host_weight_maps(gw1, gb1, gw2, gb2, mw1, mb1, mw2, mb2):
    import ml_dtypes
    f = np.float32
    bf = ml_dtypes.bfloat16
    return {
        "gw1t": np.ascontiguousarray(np.asarray(gw1, f).T.reshape(CG, 128, H).transpose(1, 0, 2).astype(bf)),
        "gb1c": np.ascontiguousarray(np.asarray(gb1, f).reshape(H, 1)),
        "gw2t": np.ascontiguousarray(np.asarray(gw2, f).T.astype(bf)),  # [H,C]
        "gb2c": np.ascontiguousarray(np.asarray(gb2, f).reshape(CG, 128).T),
        "mw1t": np.ascontiguousarray(np.asarray(mw1, f).T.reshape(CG, 128, H).transpose(1, 0, 2).astype(bf)),
        "mb1c": np.ascontiguousarray(np.asarray(mb1, f).reshape(H, 1)),
        "mw2t": np.ascontiguousarray(np.asarray(mw2, f).T),             # [H,2C]
        "mb2c": np.ascontiguousarray(np.asarray(mb2, f).reshape(2 * CG, 128).T),
        "ident": np.eye(128, dtype=f),
    }


def _run(inputs, trace=False):
    import ml_dtypes
    from concourse.bass_utils import run_bass_kernel_spmd

    nc = _get_nc()
    bf = ml_dtypes.bfloat16
    x = np.asarray(inputs["x"], np.float32).astype(bf)
    c = np.asarray(inputs["c"], np.float32).astype(bf)
    wmap = _host_weight_maps(
        inputs["gw1"], inputs["gb1"], inputs["gw2"], inputs["gb2"],
        inputs["mw1"], inputs["mb1"], inputs["mw2"], inputs["mb2"])
    in_maps = [
        dict(wmap, x=np.ascontiguousarray(x[b]), c=np.ascontiguousarray(c[b]))
        for b in range(B)
    ]
    res = run_bass_kernel_spmd(nc, in_maps, core_ids=list(range(B)), trace=trace)
    out = np.stack([np.asarray(res.results[b]["out"], np.float32) for b in range(B)],
                   axis=0)
    return out, res


def kernel(**inputs):
    out, _ = _run(inputs, trace=False)
    return out
